# revision 30
# baseline (speedup 1.0000x reference)
"""Trainium2 Bass kernel for nn_Attention (B=4, P=2048, D=768, H=12, hd=64).

Sharding: 8 cores = 4 batches x 2 half-head-groups (6 heads each).
Per core, for its (batch b, 6 heads = 3 pairs):
  - qkv projection for its heads (weights gathered host-side honoring the
    torch reshape quirk: feature (t, d, h) -> row t*768 + d*12 + h)
  - attention with scores computed transposed (sT[k, q], contraction hd=64,
    two heads of a pair run concurrently on disjoint PE row groups),
    softmax WITHOUT max subtraction (|scaled scores| <= 13.3 in log2 domain:
    the softmax scale and log2(e) are folded into the host-side qk weights so
    the kernel computes exp2), denominator via a ones-column prepended to V
    (vsb head block = [ones | 63 zeros | 64 v dims] so the PSUM rows of the
    o^T accumulation keep the denominator at partition 0 and o at 64:128)
  - o^T accumulated per chunk (pair, q-quarter) in PSUM [128, 512], normalized
    via DVE reciprocal + Pool partition-broadcast + DVE multiply
  - output projection into yT partial [768, 2048]
Host sums the two partials per batch and transposes back.

Pipeline: a single interleaved instruction stream paced by the ACT engine
(exp2 of 192 units of [128 k, 1024 q]).  The qk projection, v projection and
output projection are emitted one matmul at a time as "fill" steps in the PE
stream gaps, budgeted against modeled ACT progress so the score matmuls
(which feed ACT) never fall behind.
"""

import sys

import numpy as np

if "/opt/trn_rl_repo" not in sys.path:
    sys.path.insert(0, "/opt/trn_rl_repo")

B, P, D = 4, 2048, 768
H, HD = 12, 64
N_CORES = 8
H_LOC = 6
SCALE = HD ** -0.5
LOG2E = 1.4426950408889634
ALPHA = SCALE * LOG2E          # folded into host-side qk weights (sqrt each side)
LN2 = 0.6931471805599453

CC = 6          # contraction chunks of 128 over D=768
PT = 16         # token tiles of 128
TB = 4          # token blocks of 512
N_CHUNKS = 12   # 3 pairs x 4 q-quarters
N_UNITS = 16    # units per chunk: [128 k, 1024 q] = (kt, hd=0/1)
VW = H_LOC * 128  # vsb: per head [ones | 63 zeros | 64 v dims]

_PROG = None

_PE_COL = 0.443                 # ns per streamed column (measured)
_ACT_UNIT = (1024 + 352) / 1.2  # ns per 1024-wide ACTIVATE


def _build_program():
    import concourse.mybir as mybir
    import concourse.tile as tile
    from concourse import bacc

    f32 = mybir.dt.float32
    bf16 = mybir.dt.bfloat16
    AF = mybir.ActivationFunctionType
    AO = mybir.AluOpType

    nc = bacc.Bacc("TRN2")

    xT = nc.declare_dram_parameter("xT", [128, 4 * 6 * 512], bf16, isOutput=False)
    wqk = nc.declare_dram_parameter("wqk", [128, 6 * 768], bf16, isOutput=False)
    wv = nc.declare_dram_parameter("wv", [128, 6 * 384], bf16, isOutput=False)
    wp = nc.declare_dram_parameter("wp", [128, 3 * 768], bf16, isOutput=False)
    bqk = nc.declare_dram_parameter("bqk", [128, 6], f32, isOutput=False)
    bp = nc.declare_dram_parameter("bp", [128, 6], f32, isOutput=False)
    bvb = nc.declare_dram_parameter("bvb", [128, 384], f32, isOutput=False)
    yT = nc.declare_dram_parameter("yT", [768, 2048], f32, isOutput=True)


    ORDER = [(c % 3, c // 3) for c in range(N_CHUNKS)]  # (pair, qq) qq-major

    with tile.TileContext(nc) as tc:
        with (
            tc.tile_pool(name="persist", bufs=1) as persist,
            tc.tile_pool(name="slabring", bufs=20) as slabring,
            tc.tile_pool(name="norm", bufs=6) as norm,
            tc.tile_pool(name="psum_s", bufs=3, space="PSUM") as psum_s,
            tc.tile_pool(name="psum_o", bufs=2, space="PSUM") as psum_o,
        ):
            # ---- persistent SBUF ----
            qkt = persist.tile([128, 6, 2048], bf16, tag="qkt")
            vsb = persist.tile([128, PT, VW], bf16, tag="vsb")
            otsb = persist.tile([128, 3, 2048], bf16, tag="otsb")
            bqk_sb = persist.tile([128, 6], f32, tag="bqk_sb")
            bp_sb = persist.tile([128, 6], f32, tag="bp_sb")
            bvb_sb = persist.tile([128, 384], f32, tag="bvb_sb")
            wp_sb = persist.tile([128, 3, 768], bf16, tag="wp_sb")
            xts = persist.tile([128, 4, 6, 512], bf16, tag="xts")
            # ft-slot-major: slots [ft3, ft4, ft5, ft0, ft1, ft2], each [cc, 128]
            wqk_sb = persist.tile([128, 6, 6, 128], bf16, tag="wqk_sb")
            wv_sb = persist.tile([128, 6, 384], bf16, tag="wv_sb")
            junk = persist.tile([128, 512], bf16, tag="junk")

            # ---- DMAs: partition-major host layouts; the lead-critical
            # pieces split across the two HW DGE queues
            xts_f = xts.rearrange("p t a q -> p (t a q)")
            wqk_f = wqk_sb.rearrange("p s a f -> p (s a f)")
            nc.sync.dma_start(out=wqk_f[:, 0:768], in_=wqk[:, 0:768])
            nc.scalar.dma_start(out=xts_f[:, 0:1536], in_=xT[:, 0:1536])
            nc.sync.dma_start(out=xts_f[:, 1536:3072], in_=xT[:, 1536:3072])
            nc.scalar.dma_start(out=wqk_f[:, 2304:3072], in_=wqk[:, 2304:3072])
            nc.scalar.dma_start(out=bqk_sb, in_=bqk[:, :])
            nc.sync.dma_start(out=wqk_f[:, 768:2304], in_=wqk[:, 768:2304])
            nc.scalar.dma_start(out=xts_f[:, 3072:6144], in_=xT[:, 3072:6144])
            nc.sync.dma_start(out=wqk_f[:, 3072:4608], in_=wqk[:, 3072:4608])
            nc.sync.dma_start(out=wv_sb.rearrange("p a f -> p (a f)"),
                              in_=wv[:, :])
            nc.scalar.dma_start(out=xts_f[:, 6144:9216], in_=xT[:, 6144:9216])
            nc.sync.dma_start(out=bvb_sb, in_=bvb[:, :])
            nc.scalar.dma_start(out=xts_f[:, 9216:12288], in_=xT[:, 9216:12288])
            nc.sync.dma_start(out=wp_sb.rearrange("p a f -> p (a f)"),
                              in_=wp[:, :])
            nc.scalar.dma_start(out=bp_sb, in_=bp[:, :])

            # exp table warm + PE HAM warm during the DMA lead
            warm = norm.tile([1, 1], f32, tag="warm")
            nc.vector.memset(warm, 0.0)
            nc.scalar.activation(out=warm, in_=warm, func=AF.Exp)
            nc.vector.memset(junk, 0.0)
            jp = psum_s.tile([128, 1024], f32, tag="sp")
            for i in range(14):
                nc.tensor.matmul(jp[:, 0:512], junk[:, 0:128], junk[:, 0:512],
                                 start=(i == 0), stop=(i == 13))
            # vsb ones/zeros columns (denominator trick + partition alignment)
            nc.vector.memset(
                vsb.rearrange("p a (h g) -> p a h g", g=128)[:, :, :, 0:1], 1.0)
            nc.vector.memset(
                vsb.rearrange("p a (h g) -> p a h g", g=128)[:, :, :, 1:64], 0.0)

            # ---------------- fill job generators ----------------
            ledger = {"pe": 0.0, "v": 0}

            def pe_cost(cols):
                ledger["pe"] += cols * _PE_COL

            def qk_group(ft, tb):
                qpf = psum_s.tile([128, 1024], f32, tag="sp", name="qpf")
                qp = qpf[:, 0:512]
                for cc in range(CC):
                    slot = ft - 3 if ft >= 3 else ft + 3
                    nc.tensor.matmul(
                        qp,
                        wqk_sb[:, slot, cc, :],
                        xts[:, tb, cc, :],
                        start=(cc == 0),
                        stop=(cc == CC - 1),
                    )
                    pe_cost(512)
                    yield
                nc.vector.tensor_scalar(
                    out=qkt[:, ft, tb * 512:(tb + 1) * 512],
                    in0=qp, scalar1=bqk_sb[:, ft:ft + 1], scalar2=None,
                    op0=AO.add,
                )

            def v_job(pt):
                vpf = psum_s.tile([128, 1024], f32, tag="sp", name="vpf")
                vp = vpf[:, 0:512]
                for cc in range(CC):
                    nc.tensor.matmul(
                        vp[:, 0:384],
                        xts[:, pt // 4, cc, (pt % 4) * 128:(pt % 4 + 1) * 128],
                        wv_sb[:, cc, :],
                        start=(cc == 0),
                        stop=(cc == CC - 1),
                    )
                    pe_cost(384)
                    yield
                nc.vector.tensor_add(
                    out=vsb.rearrange("p a (h g) -> p a h g", g=128)[:, pt, :, 64:128],
                    in0=vp[:, 0:384].rearrange("p (h g) -> p h g", g=64),
                    in1=bvb_sb.rearrange("p (h g) -> p h g", g=64),
                )
                ledger["v"] = pt + 1

            def proj_job(tb, of):
                ppf = psum_s.tile([128, 1024], f32, tag="sp", name="ppf")
                pp = ppf[:, 0:512]
                for fc in range(3):
                    nc.tensor.matmul(
                        pp,
                        wp_sb[:, fc, of * 128:(of + 1) * 128],
                        otsb[:, fc, tb * 512:(tb + 1) * 512],
                        start=(fc == 0),
                        stop=(fc == 2),
                    )
                    pe_cost(512)
                    yield
                ysl = norm.tile([128, 512], f32, tag="ysl")
                nc.vector.tensor_scalar(
                    out=ysl, in0=pp, scalar1=bp_sb[:, of:of + 1], scalar2=None,
                    op0=AO.add,
                )
                nc.sync.dma_start(
                    out=yT[of * 128:(of + 1) * 128, tb * 512:(tb + 1) * 512],
                    in_=ysl,
                )

            normed = {"n": 0}  # chunks fully normalized so far

            # (due_slot, min_normed, generator) — due = deadline, budget pulls early
            FILLS = []
            for pt in range(PT):
                FILLS.append((pt + 10, 0, v_job(pt)))
            qk_sched = [(3, 3, 1), (7, 3, 2), (11, 3, 3),
                        (13, 4, 0), (14, 1, 0), (19, 4, 1), (23, 4, 2),
                        (27, 4, 3), (29, 5, 0), (30, 2, 0), (31, 5, 1),
                        (35, 5, 2), (39, 5, 3),
                        (46, 0, 1), (62, 1, 1), (78, 2, 1),
                        (94, 0, 2), (110, 1, 2), (126, 2, 2),
                        (142, 0, 3), (158, 1, 3), (174, 2, 3)]
            for due, ft, tb in qk_sched:
                FILLS.append((due, 0, qk_group(ft, tb)))
            for tb in range(3):
                for of in range(6):
                    FILLS.append(((3 * tb + 4) * N_UNITS + of * 2, 3 * tb + 3,
                                  proj_job(tb, of)))
            FILLS.sort(key=lambda x: x[0])

            # ---------------- o^T / norm ----------------
            su_tiles = {}
            ot_ps = {}
            ot_queue = []

            pending = []  # (due_slot, fn) deferred engine work

            def norm_stage0(c, hd):
                op = ot_ps[(c, hd)]
                rec = norm.tile([1, 512], f32, tag="rec")
                nc.vector.reciprocal_approx_fast(out=rec, in_=op[0:1, :])
                rb = norm.tile([64, 512], f32, tag="rb")
                nc.gpsimd.partition_broadcast(out_ap=rb, in_ap=rec, channels=64)
                return rb

            def norm_stage1(c, hd, rb):
                p, qq = ORDER[c]
                op = ot_ps.pop((c, hd))
                pb = 64 * hd
                nc.vector.tensor_mul(
                    out=otsb[pb:pb + 64, p, qq * 512:(qq + 1) * 512],
                    in0=op[64:128, :],
                    in1=rb,
                )
                if hd == 1:
                    normed["n"] = c + 1

            def norm_head(c, hd, slot):
                def s0(c=c, hd=hd):
                    rb = norm_stage0(c, hd)
                    pending.append((slot + 4, lambda: norm_stage1(c, hd, rb)))
                pending.append((slot + 2, s0))

            def ot_job(c, kc):
                """both heads' accumulation step kc for chunk c."""
                p, qq = ORDER[c]
                su = su_tiles.pop((c, kc))
                for hd in range(2):
                    key = (c, hd)
                    if key not in ot_ps:
                        ot_ps[key] = psum_o.tile([128, 512], f32, tag="op",
                                                 name=f"op{hd}")
                    ph = 2 * p + hd
                    nc.tensor.matmul(
                        ot_ps[key],
                        vsb[:, kc, ph * 128:(ph + 1) * 128],
                        su[:, hd, :],
                        start=(kc == 0),
                        stop=(kc == PT - 1),
                    )
                pe_cost(2 * 512)
                if kc == PT - 1:
                    norm_head(c, 0, cur_slot[0])
                    norm_head(c, 1, cur_slot[0])

            def pump_pending(slot):
                i = 0
                while i < len(pending):
                    due, fn = pending[i]
                    if due <= slot:
                        pending.pop(i)
                        fn()
                    else:
                        i += 1

            def pump_ot(slot, force=False):
                cur_slot[0] = slot
                while ot_queue:
                    oc, okc = ot_queue[0]
                    age = slot - (oc * N_UNITS + okc)
                    if age < 4 and not force:
                        break
                    if oc == 0 and okc >= ledger["v"]:
                        break  # v tile not emitted yet
                    if (age < 12 and not force
                            and ledger["pe"] > act[0] + 600):
                        break
                    ot_queue.pop(0)
                    ot_job(oc, okc)

            def pump_fills(slot):
                while FILLS:
                    due, min_norm, gen = FILLS[0]
                    if normed["n"] < min_norm:
                        break
                    forced = due <= slot
                    if not forced and ledger["pe"] > act[0] - 200:
                        break
                    try:
                        next(gen)
                    except StopIteration:
                        FILLS.pop(0)

            # ---------------- main pipeline ----------------
            act = [0.0]
            cur_slot = [0]
            for g in (qk_group(3, 0), qk_group(0, 0)):
                for _ in g:
                    pass

            slot = 0
            for c in range(N_CHUNKS):
                p, qq = ORDER[c]
                for u in range(N_UNITS):
                    sp = psum_s.tile([128, 1024], f32, tag="sp")
                    for hd in range(2):
                        pb = 64 * hd
                        nc.tensor.matmul(
                            sp[:, hd * 512:(hd + 1) * 512],
                            qkt[pb:pb + 64, 3 + p, u * 128:(u + 1) * 128],
                            qkt[pb:pb + 64, p, qq * 512:(qq + 1) * 512],
                            start=True,
                            stop=True,
                        )
                    pe_cost(512)  # two heads run concurrently
                    su = slabring.tile([128, 2, 512], bf16, tag="su")
                    nc.scalar.activation(
                        out=su.rearrange("p a b -> p (a b)"),
                        in_=sp[:, 0:1024],
                        func=AF.Exp,
                        scale=LN2,
                    )
                    su_tiles[(c, u)] = su
                    ot_queue.append((c, u))
                    act[0] += _ACT_UNIT
                    slot += 1

                    pump_pending(slot)
                    pump_ot(slot, force=(c == N_CHUNKS - 1 and u >= 12))
                    pump_fills(slot)

            # ---------------- tail ----------------
            pump_ot(slot, force=True)
            pump_pending(10 ** 9)
            while FILLS:
                _, _, gen = FILLS.pop(0)
                for _ in gen:
                    pass
            for of in range(6):
                for _ in proj_job(3, of):
                    pass

    nc.finalize()
    return nc


def _get_program():
    global _PROG
    if _PROG is None:
        _PROG = _build_program()
    return _PROG


def _prep_core_inputs(x, w_qkv, b_qkv, w_proj, b_proj, core):
    b, half = core // 2, core % 2
    heads = np.arange(H_LOC) + H_LOC * half  # global head ids
    d = np.arange(HD)

    import ml_dtypes
    bft = ml_dtypes.bfloat16
    # [128, 4*6*512]: tb-major then cc, partition-major rows
    xTf = x[b].T.astype(bft)                      # [768, 2048]
    xr = xTf.reshape(6, 128, 4, 512).transpose(1, 2, 0, 3)  # [128, 4, 6, 512]
    xT = np.ascontiguousarray(xr.reshape(128, 4 * 6 * 512))

    # qk feature selection honoring torch reshape quirk: row = t*768 + d*12 + h
    # feature tiles: q(0,1) q(2,3) q(4,5) k(0,1) k(2,3) k(4,5)
    qk_rows = np.empty(768, np.int64)
    for j in range(3):
        for hp in range(2):
            hh = heads[2 * j + hp]
            base = j * 128 + hp * 64
            qk_rows[base:base + 64] = d * 12 + hh
            qk_rows[384 + base:384 + base + 64] = 768 + d * 12 + hh
    ra = np.sqrt(ALPHA)
    wqk_f = w_qkv[qk_rows] * ra          # fold sqrt(scale*log2e) into q AND k
    bqk_f = b_qkv[qk_rows] * ra
    wqk_c = wqk_f.T.astype(bft)                   # [768 c, 768 feat]
    # [128, slot(ft 3,4,5,0,1,2), cc, 128] partition-major
    wr = wqk_c.reshape(6, 128, 6, 128)            # [cc, p, ft, 128]
    wr = wr[:, :, [3, 4, 5, 0, 1, 2], :]          # slot order
    wqk_t = np.ascontiguousarray(
        wr.transpose(1, 2, 0, 3).reshape(128, 6 * 6 * 128))
    bqk_t = np.ascontiguousarray(bqk_f.reshape(6, 128).T)  # [128, 6]

    wv_np = np.empty((768, 384), np.float64)
    bv_np = np.empty(384, np.float64)
    for i in range(H_LOC):
        rows = 1536 + d * 12 + heads[i]
        wv_np[:, 64 * i:64 * i + 64] = w_qkv[rows].T
        bv_np[64 * i:64 * i + 64] = b_qkv[rows]
    wv_c = wv_np.astype(bft)                      # [768, 384]
    wv_t = np.ascontiguousarray(
        wv_c.reshape(6, 128, 384).transpose(1, 0, 2).reshape(128, 6 * 384))
    bvb = np.ascontiguousarray(
        np.broadcast_to(bv_np.astype(np.float32), (128, 384)))

    wp_c = np.empty((384, 768), bft)
    for i in range(H_LOC):
        cols = 64 * heads[i] + d
        wp_c[64 * i:64 * i + 64] = w_proj[:, cols].T
    wp_t = np.ascontiguousarray(
        wp_c.reshape(3, 128, 768).transpose(1, 0, 2).reshape(128, 3 * 768))
    bp_t = np.ascontiguousarray((b_proj * 0.5).reshape(6, 128).T)

    return {
        "xT": xT,
        "wqk": wqk_t,
        "wv": wv_t,
        "wp": np.ascontiguousarray(wp_t),
        "bqk": bqk_t,
        "bp": np.ascontiguousarray(bp_t),
        "bvb": bvb,
    }


def _run(inputs, trace=False, **kw):
    from concourse.bass_utils import run_bass_kernel_spmd

    nc = _get_program()
    x = np.asarray(inputs["x"], np.float32)
    w_qkv = np.asarray(inputs["w_qkv"], np.float64)
    b_qkv = np.asarray(inputs["b_qkv"], np.float64)
    w_proj = np.asarray(inputs["w_proj"], np.float64)
    b_proj = np.asarray(inputs["b_proj"], np.float64)

    in_maps = [
        _prep_core_inputs(x, w_qkv, b_qkv, w_proj, b_proj, c)
        for c in range(N_CORES)
    ]
    res = run_bass_kernel_spmd(nc, in_maps, list(range(N_CORES)),
                               trace=trace, **kw)

    out = np.empty((B, P, D), np.float32)
    for b in range(B):
        yt = res.results[2 * b]["yT"] + res.results[2 * b + 1]["yT"]
        out[b] = yt.T
    return out, res


def kernel(**inputs):
    out, _ = _run(inputs)
    return out


# revision 31
# speedup vs baseline: 1.0116x; 1.0116x over previous
"""Trainium2 Bass kernel for nn_Attention (B=4, P=2048, D=768, H=12, hd=64).

Sharding: 8 cores = 4 batches x 2 half-head-groups (6 heads each).
Per core, for its (batch b, 6 heads = 3 pairs):
  - qkv projection for its heads (weights gathered host-side honoring the
    torch reshape quirk: feature (t, d, h) -> row t*768 + d*12 + h)
  - attention with scores computed transposed (sT[k, q], contraction hd=64,
    two heads of a pair run concurrently on disjoint PE row groups),
    softmax WITHOUT max subtraction (|scaled scores| <= 13.3 in log2 domain:
    the softmax scale and log2(e) are folded into the host-side qk weights so
    the kernel computes exp2), denominator via a ones-column prepended to V
    (vsb head block = [ones | 63 zeros | 64 v dims] so the PSUM rows of the
    o^T accumulation keep the denominator at partition 0 and o at 64:128)
  - o^T accumulated per chunk (pair, q-quarter) in PSUM [128, 512], normalized
    via DVE reciprocal + Pool partition-broadcast + DVE multiply
  - output projection into yT partial [768, 2048]
Host sums the two partials per batch and transposes back.

Pipeline: a single interleaved instruction stream paced by the ACT engine
(exp2 of 192 units of [128 k, 1024 q]).  The qk projection, v projection and
output projection are emitted one matmul at a time as "fill" steps in the PE
stream gaps, budgeted against modeled ACT progress so the score matmuls
(which feed ACT) never fall behind.
"""

import sys

import numpy as np

if "/opt/trn_rl_repo" not in sys.path:
    sys.path.insert(0, "/opt/trn_rl_repo")

B, P, D = 4, 2048, 768
H, HD = 12, 64
N_CORES = 8
H_LOC = 6
SCALE = HD ** -0.5
LOG2E = 1.4426950408889634
ALPHA = SCALE * LOG2E          # folded into host-side qk weights (sqrt each side)
LN2 = 0.6931471805599453

CC = 6          # contraction chunks of 128 over D=768
PT = 16         # token tiles of 128
TB = 4          # token blocks of 512
N_CHUNKS = 12   # 3 pairs x 4 q-quarters
N_UNITS = 16    # units per chunk: [128 k, 1024 q] = (kt, hd=0/1)
VW = H_LOC * 128  # vsb: per head [ones | 63 zeros | 64 v dims]

_PROG = None

_PE_COL = 0.443                 # ns per streamed column (measured)
_ACT_UNIT = (1024 + 352) / 1.2  # ns per 1024-wide ACTIVATE


def _build_program():
    import concourse.mybir as mybir
    import concourse.tile as tile
    from concourse import bacc

    f32 = mybir.dt.float32
    bf16 = mybir.dt.bfloat16
    AF = mybir.ActivationFunctionType
    AO = mybir.AluOpType

    nc = bacc.Bacc("TRN2")

    xT = nc.declare_dram_parameter("xT", [128, 4 * 6 * 512], bf16, isOutput=False)
    wqk = nc.declare_dram_parameter("wqk", [128, 6 * 768], bf16, isOutput=False)
    wv = nc.declare_dram_parameter("wv", [128, 6 * 384], bf16, isOutput=False)
    wp = nc.declare_dram_parameter("wp", [128, 3 * 768], bf16, isOutput=False)
    bqk = nc.declare_dram_parameter("bqk", [128, 6], f32, isOutput=False)
    bp = nc.declare_dram_parameter("bp", [128, 6], f32, isOutput=False)
    bvb = nc.declare_dram_parameter("bvb", [128, 384], f32, isOutput=False)
    yT = nc.declare_dram_parameter("yT", [768, 2048], bf16, isOutput=True)


    ORDER = [(c % 3, c // 3) for c in range(N_CHUNKS)]  # (pair, qq) qq-major

    with tile.TileContext(nc) as tc:
        with (
            tc.tile_pool(name="persist", bufs=1) as persist,
            tc.tile_pool(name="slabring", bufs=20) as slabring,
            tc.tile_pool(name="norm", bufs=6) as norm,
            tc.tile_pool(name="psum_s", bufs=3, space="PSUM") as psum_s,
            tc.tile_pool(name="psum_o", bufs=2, space="PSUM") as psum_o,
        ):
            # ---- persistent SBUF ----
            qkt = persist.tile([128, 6, 2048], bf16, tag="qkt")
            vsb = persist.tile([128, PT, VW], bf16, tag="vsb")
            otsb = persist.tile([128, 3, 2048], bf16, tag="otsb")
            bqk_sb = persist.tile([128, 6], f32, tag="bqk_sb")
            bp_sb = persist.tile([128, 6], f32, tag="bp_sb")
            bvb_sb = persist.tile([128, 384], f32, tag="bvb_sb")
            wp_sb = persist.tile([128, 3, 768], bf16, tag="wp_sb")
            xts = persist.tile([128, 4, 6, 512], bf16, tag="xts")
            # ft-slot-major: slots [ft3, ft4, ft5, ft0, ft1, ft2], each [cc, 128]
            wqk_sb = persist.tile([128, 6, 6, 128], bf16, tag="wqk_sb")
            wv_sb = persist.tile([128, 6, 384], bf16, tag="wv_sb")
            junk = persist.tile([128, 512], bf16, tag="junk")

            # ---- DMAs: partition-major host layouts; the lead-critical
            # pieces split across the two HW DGE queues
            xts_f = xts.rearrange("p t a q -> p (t a q)")
            wqk_f = wqk_sb.rearrange("p s a f -> p (s a f)")
            nc.sync.dma_start(out=wqk_f[:, 0:768], in_=wqk[:, 0:768])
            nc.scalar.dma_start(out=xts_f[:, 0:1536], in_=xT[:, 0:1536])
            nc.sync.dma_start(out=xts_f[:, 1536:3072], in_=xT[:, 1536:3072])
            nc.scalar.dma_start(out=wqk_f[:, 2304:3072], in_=wqk[:, 2304:3072])
            nc.scalar.dma_start(out=bqk_sb, in_=bqk[:, :])
            nc.sync.dma_start(out=wqk_f[:, 768:2304], in_=wqk[:, 768:2304])
            nc.scalar.dma_start(out=xts_f[:, 3072:6144], in_=xT[:, 3072:6144])
            nc.sync.dma_start(out=wqk_f[:, 3072:4608], in_=wqk[:, 3072:4608])
            nc.sync.dma_start(out=wv_sb.rearrange("p a f -> p (a f)"),
                              in_=wv[:, :])
            nc.scalar.dma_start(out=xts_f[:, 6144:9216], in_=xT[:, 6144:9216])
            nc.sync.dma_start(out=bvb_sb, in_=bvb[:, :])
            nc.scalar.dma_start(out=xts_f[:, 9216:12288], in_=xT[:, 9216:12288])
            nc.sync.dma_start(out=wp_sb.rearrange("p a f -> p (a f)"),
                              in_=wp[:, :])
            nc.scalar.dma_start(out=bp_sb, in_=bp[:, :])

            # exp table warm + PE HAM warm during the DMA lead
            warm = norm.tile([1, 1], f32, tag="warm")
            nc.vector.memset(warm, 0.0)
            nc.scalar.activation(out=warm, in_=warm, func=AF.Exp)
            nc.vector.memset(junk, 0.0)
            jp = psum_s.tile([128, 1024], f32, tag="sp")
            for i in range(14):
                nc.tensor.matmul(jp[:, 0:512], junk[:, 0:128], junk[:, 0:512],
                                 start=(i == 0), stop=(i == 13))
            # vsb ones/zeros columns (denominator trick + partition alignment)
            nc.vector.memset(
                vsb.rearrange("p a (h g) -> p a h g", g=128)[:, :, :, 0:1], 1.0)
            nc.vector.memset(
                vsb.rearrange("p a (h g) -> p a h g", g=128)[:, :, :, 1:64], 0.0)

            # ---------------- fill job generators ----------------
            ledger = {"pe": 0.0, "v": 0}

            def pe_cost(cols):
                ledger["pe"] += cols * _PE_COL

            def qk_group(ft, tb):
                qpf = psum_s.tile([128, 1024], f32, tag="sp", name="qpf")
                qp = qpf[:, 0:512]
                for cc in range(CC):
                    slot = ft - 3 if ft >= 3 else ft + 3
                    nc.tensor.matmul(
                        qp,
                        wqk_sb[:, slot, cc, :],
                        xts[:, tb, cc, :],
                        start=(cc == 0),
                        stop=(cc == CC - 1),
                    )
                    pe_cost(512)
                    yield
                nc.vector.tensor_scalar(
                    out=qkt[:, ft, tb * 512:(tb + 1) * 512],
                    in0=qp, scalar1=bqk_sb[:, ft:ft + 1], scalar2=None,
                    op0=AO.add,
                )

            def v_job(pt):
                vpf = psum_s.tile([128, 1024], f32, tag="sp", name="vpf")
                vp = vpf[:, 0:512]
                for cc in range(CC):
                    nc.tensor.matmul(
                        vp[:, 0:384],
                        xts[:, pt // 4, cc, (pt % 4) * 128:(pt % 4 + 1) * 128],
                        wv_sb[:, cc, :],
                        start=(cc == 0),
                        stop=(cc == CC - 1),
                    )
                    pe_cost(384)
                    yield
                nc.vector.tensor_add(
                    out=vsb.rearrange("p a (h g) -> p a h g", g=128)[:, pt, :, 64:128],
                    in0=vp[:, 0:384].rearrange("p (h g) -> p h g", g=64),
                    in1=bvb_sb.rearrange("p (h g) -> p h g", g=64),
                )
                ledger["v"] = pt + 1

            def proj_job(tb, of):
                ppf = psum_s.tile([128, 1024], f32, tag="sp", name="ppf")
                pp = ppf[:, 0:512]
                for fc in range(3):
                    nc.tensor.matmul(
                        pp,
                        wp_sb[:, fc, of * 128:(of + 1) * 128],
                        otsb[:, fc, tb * 512:(tb + 1) * 512],
                        start=(fc == 0),
                        stop=(fc == 2),
                    )
                    pe_cost(512)
                    yield
                ysl = norm.tile([128, 512], bf16, tag="ysl")
                nc.vector.tensor_scalar(
                    out=ysl, in0=pp, scalar1=bp_sb[:, of:of + 1], scalar2=None,
                    op0=AO.add,
                )
                nc.sync.dma_start(
                    out=yT[of * 128:(of + 1) * 128, tb * 512:(tb + 1) * 512],
                    in_=ysl,
                )

            normed = {"n": 0}  # chunks fully normalized so far

            # (due_slot, min_normed, generator) — due = deadline, budget pulls early
            FILLS = []
            for pt in range(PT):
                FILLS.append((pt + 10, 0, v_job(pt)))
            qk_sched = [(3, 3, 1), (7, 3, 2), (11, 3, 3),
                        (13, 4, 0), (14, 1, 0), (19, 4, 1), (23, 4, 2),
                        (27, 4, 3), (29, 5, 0), (30, 2, 0), (31, 5, 1),
                        (35, 5, 2), (39, 5, 3),
                        (46, 0, 1), (62, 1, 1), (78, 2, 1),
                        (94, 0, 2), (110, 1, 2), (126, 2, 2),
                        (142, 0, 3), (158, 1, 3), (174, 2, 3)]
            for due, ft, tb in qk_sched:
                FILLS.append((due, 0, qk_group(ft, tb)))
            for tb in range(3):
                for of in range(6):
                    FILLS.append(((3 * tb + 4) * N_UNITS + of * 2, 3 * tb + 3,
                                  proj_job(tb, of)))
            FILLS.sort(key=lambda x: x[0])

            # ---------------- o^T / norm ----------------
            su_tiles = {}
            ot_ps = {}
            ot_queue = []

            pending = []  # (due_slot, fn) deferred engine work

            def norm_stage0(c, hd):
                op = ot_ps[(c, hd)]
                rec = norm.tile([1, 512], f32, tag="rec")
                nc.vector.reciprocal_approx_fast(out=rec, in_=op[0:1, :])
                rb = norm.tile([64, 512], f32, tag="rb")
                nc.gpsimd.partition_broadcast(out_ap=rb, in_ap=rec, channels=64)
                return rb

            def norm_stage1(c, hd, rb):
                p, qq = ORDER[c]
                op = ot_ps.pop((c, hd))
                pb = 64 * hd
                nc.vector.tensor_mul(
                    out=otsb[pb:pb + 64, p, qq * 512:(qq + 1) * 512],
                    in0=op[64:128, :],
                    in1=rb,
                )
                if hd == 1:
                    normed["n"] = c + 1

            def norm_head(c, hd, slot):
                def s0(c=c, hd=hd):
                    rb = norm_stage0(c, hd)
                    pending.append((slot + 4, lambda: norm_stage1(c, hd, rb)))
                pending.append((slot + 2, s0))

            def ot_job(c, kc):
                """both heads' accumulation step kc for chunk c."""
                p, qq = ORDER[c]
                su = su_tiles.pop((c, kc))
                for hd in range(2):
                    key = (c, hd)
                    if key not in ot_ps:
                        ot_ps[key] = psum_o.tile([128, 512], f32, tag="op",
                                                 name=f"op{hd}")
                    ph = 2 * p + hd
                    nc.tensor.matmul(
                        ot_ps[key],
                        vsb[:, kc, ph * 128:(ph + 1) * 128],
                        su[:, hd, :],
                        start=(kc == 0),
                        stop=(kc == PT - 1),
                    )
                pe_cost(2 * 512)
                if kc == PT - 1:
                    norm_head(c, 0, cur_slot[0])
                    norm_head(c, 1, cur_slot[0])

            def pump_pending(slot):
                i = 0
                while i < len(pending):
                    due, fn = pending[i]
                    if due <= slot:
                        pending.pop(i)
                        fn()
                    else:
                        i += 1

            def pump_ot(slot, force=False):
                cur_slot[0] = slot
                while ot_queue:
                    oc, okc = ot_queue[0]
                    age = slot - (oc * N_UNITS + okc)
                    if age < 4 and not force:
                        break
                    if oc == 0 and okc >= ledger["v"]:
                        break  # v tile not emitted yet
                    if (age < 12 and not force
                            and ledger["pe"] > act[0] + 600):
                        break
                    ot_queue.pop(0)
                    ot_job(oc, okc)

            def pump_fills(slot):
                while FILLS:
                    due, min_norm, gen = FILLS[0]
                    if normed["n"] < min_norm:
                        break
                    forced = due <= slot
                    if not forced and ledger["pe"] > act[0] - 200:
                        break
                    try:
                        next(gen)
                    except StopIteration:
                        FILLS.pop(0)

            # ---------------- main pipeline ----------------
            act = [0.0]
            cur_slot = [0]
            for g in (qk_group(3, 0), qk_group(0, 0)):
                for _ in g:
                    pass

            slot = 0
            for c in range(N_CHUNKS):
                p, qq = ORDER[c]
                for u in range(N_UNITS):
                    sp = psum_s.tile([128, 1024], f32, tag="sp")
                    for hd in range(2):
                        pb = 64 * hd
                        nc.tensor.matmul(
                            sp[:, hd * 512:(hd + 1) * 512],
                            qkt[pb:pb + 64, 3 + p, u * 128:(u + 1) * 128],
                            qkt[pb:pb + 64, p, qq * 512:(qq + 1) * 512],
                            start=True,
                            stop=True,
                        )
                    pe_cost(512)  # two heads run concurrently
                    su = slabring.tile([128, 2, 512], bf16, tag="su")
                    nc.scalar.activation(
                        out=su.rearrange("p a b -> p (a b)"),
                        in_=sp[:, 0:1024],
                        func=AF.Exp,
                        scale=LN2,
                    )
                    su_tiles[(c, u)] = su
                    ot_queue.append((c, u))
                    act[0] += _ACT_UNIT
                    slot += 1

                    pump_pending(slot)
                    pump_ot(slot, force=(c == N_CHUNKS - 1 and u >= 12))
                    pump_fills(slot)

            # ---------------- tail ----------------
            pump_ot(slot, force=True)
            pump_pending(10 ** 9)
            while FILLS:
                _, _, gen = FILLS.pop(0)
                for _ in gen:
                    pass
            for of in range(6):
                for _ in proj_job(3, of):
                    pass

    nc.finalize()
    return nc


def _get_program():
    global _PROG
    if _PROG is None:
        _PROG = _build_program()
    return _PROG


def _prep_core_inputs(x, w_qkv, b_qkv, w_proj, b_proj, core):
    b, half = core // 2, core % 2
    heads = np.arange(H_LOC) + H_LOC * half  # global head ids
    d = np.arange(HD)

    import ml_dtypes
    bft = ml_dtypes.bfloat16
    # [128, 4*6*512]: tb-major then cc, partition-major rows
    xTf = x[b].T.astype(bft)                      # [768, 2048]
    xr = xTf.reshape(6, 128, 4, 512).transpose(1, 2, 0, 3)  # [128, 4, 6, 512]
    xT = np.ascontiguousarray(xr.reshape(128, 4 * 6 * 512))

    # qk feature selection honoring torch reshape quirk: row = t*768 + d*12 + h
    # feature tiles: q(0,1) q(2,3) q(4,5) k(0,1) k(2,3) k(4,5)
    qk_rows = np.empty(768, np.int64)
    for j in range(3):
        for hp in range(2):
            hh = heads[2 * j + hp]
            base = j * 128 + hp * 64
            qk_rows[base:base + 64] = d * 12 + hh
            qk_rows[384 + base:384 + base + 64] = 768 + d * 12 + hh
    ra = np.sqrt(ALPHA)
    wqk_f = w_qkv[qk_rows] * ra          # fold sqrt(scale*log2e) into q AND k
    bqk_f = b_qkv[qk_rows] * ra
    wqk_c = wqk_f.T.astype(bft)                   # [768 c, 768 feat]
    # [128, slot(ft 3,4,5,0,1,2), cc, 128] partition-major
    wr = wqk_c.reshape(6, 128, 6, 128)            # [cc, p, ft, 128]
    wr = wr[:, :, [3, 4, 5, 0, 1, 2], :]          # slot order
    wqk_t = np.ascontiguousarray(
        wr.transpose(1, 2, 0, 3).reshape(128, 6 * 6 * 128))
    bqk_t = np.ascontiguousarray(bqk_f.reshape(6, 128).T)  # [128, 6]

    wv_np = np.empty((768, 384), np.float64)
    bv_np = np.empty(384, np.float64)
    for i in range(H_LOC):
        rows = 1536 + d * 12 + heads[i]
        wv_np[:, 64 * i:64 * i + 64] = w_qkv[rows].T
        bv_np[64 * i:64 * i + 64] = b_qkv[rows]
    wv_c = wv_np.astype(bft)                      # [768, 384]
    wv_t = np.ascontiguousarray(
        wv_c.reshape(6, 128, 384).transpose(1, 0, 2).reshape(128, 6 * 384))
    bvb = np.ascontiguousarray(
        np.broadcast_to(bv_np.astype(np.float32), (128, 384)))

    wp_c = np.empty((384, 768), bft)
    for i in range(H_LOC):
        cols = 64 * heads[i] + d
        wp_c[64 * i:64 * i + 64] = w_proj[:, cols].T
    wp_t = np.ascontiguousarray(
        wp_c.reshape(3, 128, 768).transpose(1, 0, 2).reshape(128, 3 * 768))
    bp_t = np.ascontiguousarray((b_proj * 0.5).reshape(6, 128).T)

    return {
        "xT": xT,
        "wqk": wqk_t,
        "wv": wv_t,
        "wp": np.ascontiguousarray(wp_t),
        "bqk": bqk_t,
        "bp": np.ascontiguousarray(bp_t),
        "bvb": bvb,
    }


def _run(inputs, trace=False, **kw):
    from concourse.bass_utils import run_bass_kernel_spmd

    nc = _get_program()
    x = np.asarray(inputs["x"], np.float32)
    w_qkv = np.asarray(inputs["w_qkv"], np.float64)
    b_qkv = np.asarray(inputs["b_qkv"], np.float64)
    w_proj = np.asarray(inputs["w_proj"], np.float64)
    b_proj = np.asarray(inputs["b_proj"], np.float64)

    in_maps = [
        _prep_core_inputs(x, w_qkv, b_qkv, w_proj, b_proj, c)
        for c in range(N_CORES)
    ]
    res = run_bass_kernel_spmd(nc, in_maps, list(range(N_CORES)),
                               trace=trace, **kw)

    out = np.empty((B, P, D), np.float32)
    for b in range(B):
        yt = (res.results[2 * b]["yT"].astype(np.float32)
              + res.results[2 * b + 1]["yT"].astype(np.float32))
        out[b] = yt.T
    return out, res


def kernel(**inputs):
    out, _ = _run(inputs)
    return out
